# revision 57
# baseline (speedup 1.0000x reference)
"""Pointer-generator head on 8 Trainium2 NeuronCores (Bass/Tile).

Computation (per batch row b):
    p_gen = sigmoid(context @ w_c + state @ w_s + emb @ w_y + b)
    out   = p_gen * vocab_dist
    out[b, src_ids[b, t]] += (1 - p_gen) * attn_dist[b, t]   (masked, clamped)

Sharding: batch dim (512) split across 8 cores, 64 rows each; every core keeps
its rows' full V=32000 columns so the scatter-add stays core-local; the small
parameter vectors are replicated.

Layout (per core; partition p = 2b+h holds row b's half-row h contiguously):

  * bf16 streaming: vocab_dist is host-converted to bf16 and the output is
    produced in bf16 (host upconverts) — halves the dominant HBM traffic.
    Measured rel err ~1.05e-2 vs the 2e-2 gate.
  * p_gen: 20 accumulating fp16 PE matmuls on a non-duplicated [128, 64]
    transposed activation layout produce [64,1] dots; (1 - p_gen) comes
    directly from sigmoid(-z - b) on the scalar engine, and a tiny constant
    0/1 matmul broadcasts it to the interleaved [128,1] PSUM layout.
  * scatter: the host buckets each partition's items into NB=8 buckets of
    2000 bf16 columns and window-packs duplicate-id groups (adaptive window
    W = max duplicate multiplicity). The device computes windowed duplicate-
    group sums with a 16-bit equality mask (DVE at 2x rate, split in two
    halves), scales them by (1 - p_gen) fused with the bf16 cast on the
    scalar engine, then 8 GPSIMD local_scatter calls build bf16 sparse
    tiles (one int16 index per value; non-first group members and padding
    carry -1 and are dropped). A dummy scatter at program start absorbs the
    ~2us ucode launch cost.
  * dense: all 4 stream chunks (bucket split 2/3/2/1 so the tail chunk is
    small) are resident in SBUF with every input DMA issued up front; the
    scalar engine pre-scales each chunk by p_gen in place, and one plain
    all-bf16 DVE tensor_tensor add per chunk (2 elem/cycle mode) folds in
    the sparse tile before the store.

Host-side work is limited to index metadata (permutations, bucket indices),
pure data relayout (sharding, transposition) and dtype conversion.
"""

import numpy as np

import concourse.bacc as bacc
import concourse.mybir as mybir
import concourse.tile as tile
from concourse import bass_utils

# ---- problem shape (hardcoded per spec) ----
B = 512
T = 400
V = 32000
ENC, HID, EMB = 1024, 1024, 512
NCORES = 8

P = 128
BSH = B // NCORES       # 64 rows per core
HV = V // 2             # half-row width per partition (16000)
D = ENC + HID + EMB     # 2560
NK = D // P             # 20 K-chunks for the p_gen matmul
NB = 8                  # scatter buckets per partition
BW = HV // NB           # 2000 bf16 columns per bucket
CHUNK_BUCKETS = (2, 3, 2, 1)   # buckets per stream chunk (small tail chunk)
NSTREAM = len(CHUNK_BUCKETS)

F32 = mybir.dt.float32
BF16 = mybir.dt.bfloat16
FP16 = mybir.dt.float16
I16 = mybir.dt.int16
NP_BF16 = mybir.dt.np(mybir.dt.bfloat16)


# --------------------------------------------------------------------------
# host-side index prep (pure metadata / relayout)
# --------------------------------------------------------------------------

def _bucketize(src_ids: np.ndarray, vocab_size: int):
    """Bucket one shard's unmasked items by (partition, bucket).

    Returns buckets[p][c] = list of (bucket-local target, [t indices]) groups;
    each group shares one raw id.  Masked items (id >= min(vocab_size, V))
    contribute nothing and are dropped.
    """
    id_lim = min(int(vocab_size), V)
    buckets = [[[] for _ in range(NB)] for _ in range(P)]
    for b in range(BSH):
        order: dict[int, list[int]] = {}
        for t, i in enumerate(src_ids[b].tolist()):
            if i < id_lim:
                order.setdefault(i, []).append(t)
        for i, g in order.items():
            h, off = divmod(i, HV)
            c = off // BW
            buckets[2 * b + h][c].append((off - c * BW, g))
    return buckets


def _pack_bucket(groups, w: int):
    """First-fit-decreasing of duplicate-id groups into windows of w.

    Returns (placements, nwin); placements is a list of
    (slot_offset_within_segment, loc, [t indices])."""
    fills: list[int] = []
    placements = []
    for loc, ts in sorted(groups, key=lambda g: -len(g[1])):
        if len(ts) > w:
            raise ValueError(f"duplicate group of {len(ts)} exceeds window {w}")
        for wdx, f in enumerate(fills):
            if f + len(ts) <= w:
                break
        else:
            wdx = len(fills)
            fills.append(0)
        placements.append((wdx * w + fills[wdx], loc, ts))
        fills[wdx] += len(ts)
    return placements, len(fills)


def _prep_shard(attn: np.ndarray, src_ids: np.ndarray, vocab_size: int,
                s: int, w: int):
    """-> (attn_p, ids16, lsidx): [P, NB*s] bf16, [P, NB*s] i16, [P, NB*s] i16.

    Slot layout per partition: NB segments of s slots (s//w windows each);
    every duplicate-id group occupies consecutive slots inside one window.
    lsidx holds one int16 bucket-local bf16 column index per slot at the
    group's first member; everything else is -1 (dropped)."""
    TS = NB * s
    attn_p = np.zeros((P, TS), NP_BF16)
    ids16 = np.full((P, TS), -1, np.int16)
    lsidx = np.full((P, TS), -1, np.int16)
    buckets = _bucketize(src_ids, vocab_size)
    for p in range(P):
        row = p // 2
        for c in range(NB):
            placements, nwin = _pack_bucket(buckets[p][c], w)
            assert nwin * w <= s
            for slot, loc, ts in placements:
                j = c * s + slot
                gid = src_ids[row, ts[0]]
                for k, t in enumerate(ts):
                    attn_p[p, j + k] = attn[row, t]
                    ids16[p, j + k] = gid
                lsidx[p, j] = loc
    return attn_p, ids16, lsidx


def _slot_requirement(src_ids_full: np.ndarray, vocab_size: int):
    """Global (S, W): W = max duplicate-group size (>=2), S = max windows *
    W over any (core, partition, bucket), rounded so S is even and S % W == 0."""
    all_buckets = [
        _bucketize(src_ids_full[c * BSH : (c + 1) * BSH], vocab_size)
        for c in range(NCORES)
    ]
    w = 2
    for buckets in all_buckets:
        for p in range(P):
            for ch in range(NB):
                for _, ts in buckets[p][ch]:
                    w = max(w, len(ts))
    nwin = 1
    for buckets in all_buckets:
        for p in range(P):
            for ch in range(NB):
                _, n = _pack_bucket(buckets[p][ch], w)
                nwin = max(nwin, n)
    if (nwin * w) % 2:
        nwin += 1
    return nwin * w, w


# --------------------------------------------------------------------------
# device kernel (per core; SPMD across 8 cores)
# --------------------------------------------------------------------------

def _build_kernel(tc: tile.TileContext, out, ins, b_const: float, S: int,
                  W: int):
    nc = tc.nc
    vd, xt_in, wt_in, dup_in, meta_in = ins
    TS = NB * S
    NWT = TS // W

    with tc.tile_pool(name="res", bufs=1) as sp, \
         tc.tile_pool(name="psum", bufs=1, space="PSUM") as pp:
        # ---- metadata DMAs first (small; unblock compute engines). All on
        # one dispatch queue: transfers drain roughly in dispatch order, so
        # small critical tensors go before the big vocab_dist chunks. The
        # three [P, TS] scatter-metadata tensors (ids, attn bit-cast, slot
        # indices) ship as ONE i16 tensor to save dispatch slots ----
        meta = sp.tile([P, 3 * TS], I16)
        nc.sync.dma_start(meta[:], meta_in[:, :])
        wt = sp.tile([P, NK], FP16)
        nc.sync.dma_start(wt[:], wt_in[:, :])
        XH = NK * BSH // 2
        xTa = sp.tile([P, XH], FP16)
        nc.sync.dma_start(xTa[:], xt_in[:, :XH])
        xTb = sp.tile([P, XH], FP16)
        nc.sync.dma_start(xTb[:], xt_in[:, XH:])
        dup = sp.tile([BSH, P], F32)
        nc.sync.dma_start(dup[:], dup_in[:, :])

        # ---- dummy GPSIMD scatter: absorbs the ~2us LocalScatter ucode
        # launch cost while the metadata DMAs are still in flight ----
        dmy_d = sp.tile([P, 4], BF16)
        nc.gpsimd.memset(dmy_d[:], 0)
        dmy_i = sp.tile([P, 4], I16)
        nc.gpsimd.memset(dmy_i[:], -1)
        dmy_o = sp.tile([P, 4], BF16)
        nc.gpsimd.local_scatter(
            out_ap=dmy_o[:], data_ap=dmy_d[:], idxs_ap=dmy_i[:],
            channels=P, num_elems=4, num_idxs=4,
        )

        # ---- all stream-chunk input DMAs issued up front (resident) ----
        vdv = vd.rearrange("(p v) -> p v", p=P)
        outv = out.rearrange("(p v) -> p v", p=P)
        tls = []
        off = 0
        for c, nbk in enumerate(CHUNK_BUCKETS):
            cw = nbk * BW
            tl = sp.tile([P, cw], BF16, name=f"tl{c}")
            nc.sync.dma_start(tl[:], vdv[:, off : off + cw])
            tls.append((tl, off, cw))
            off += cw

        # ---- windowed duplicate-group sums (16-bit DVE ops), split into
        # two halves so the first half is ready as early as possible; the
        # p_gen matmul block rides the PE between the halves ----
        TH = TS // 2
        NWH = NWT // 2
        NBH = NB // 2  # buckets per gs half
        eq = sp.tile([P, NWH * W * W], BF16)
        eqv = eq[:].rearrange("p (w i j) -> p w i j", i=W, j=W)
        # separate tiles per half so the first scatters depend only on half A
        gss = [sp.tile([P, TH], F32, name=f"gs{h}") for h in range(2)]
        gsbs = [sp.tile([P, TH], BF16, name=f"gsb{h}") for h in range(2)]
        # chunk bookkeeping: spt tiles and the (chunk, sub) slot of each bucket
        spts = []
        bucket_slot = []
        for c, nbk in enumerate(CHUNK_BUCKETS):
            spts.append(sp.tile([P, nbk * BW], BF16, name=f"spt{c}"))
            for sub in range(nbk):
                bucket_slot.append((c, sub))
        # p_gen matmuls interleave with the eq halves on independent engines
        dots64 = pp.tile([BSH, 1], F32, space="PSUM")
        omd64 = sp.tile([BSH, 1], F32)
        omdp = pp.tile([P, 1], F32, space="PSUM")
        for h in range(2):
            hs = slice(h * TH, (h + 1) * TH)
            idw = meta[:, hs].rearrange("p (w i) -> p w i", i=W)
            id_i = idw[:, :, :, None].to_broadcast([P, NWH, W, W])
            id_j = idw[:, :, None, :].to_broadcast([P, NWH, W, W])
            nc.vector.tensor_tensor(
                eqv, id_i, id_j, op=mybir.AluOpType.is_equal
            )
            at_j = (
                meta[:, TS + h * TH : TS + (h + 1) * TH]
                .bitcast(BF16)
                .rearrange("p (w i) -> p w i", i=W)[:, :, None, :]
                .to_broadcast([P, NWH, W, W])
            )
            nc.vector.tensor_mul(eqv, eqv, at_j)
            nc.vector.reduce_sum(
                gss[h][:].rearrange("p (w i) -> p w i", i=W),
                eqv,
                axis=mybir.AxisListType.X,
            )
            if h == 0:
                # ---- p_gen: 20 accumulating fp16 PE matmuls -> [64,1];
                # omd = 1 - sigmoid(z + b) = sigmoid(-z - b) directly on the
                # scalar engine, then a constant 0/1 matmul broadcasts it to
                # the interleaved [128,1] PSUM layout ----
                for k in range(NK):
                    xh = (xTa, xTb)[k >= NK // 2]
                    kk = k % (NK // 2)
                    nc.tensor.matmul(
                        dots64[:],
                        lhsT=xh[:, kk * BSH : (kk + 1) * BSH],
                        rhs=wt[:, k : k + 1],
                        start=(k == 0),
                        stop=(k == NK - 1),
                    )
                nc.scalar.activation(
                    omd64[:], dots64[:],
                    mybir.ActivationFunctionType.Sigmoid,
                    scale=-1.0, bias=-b_const,
                )
                nc.tensor.matmul(omdp[:], lhsT=dup[:], rhs=omd64[:],
                                 start=True, stop=True)

        omd = sp.tile([P, 1], F32)  # activation scale must come from SBUF
        nc.scalar.activation(
            omd[:], omdp[:], mybir.ActivationFunctionType.Copy
        )
        # scale group sums by (1 - p_gen) fused with the bf16 cast (scalar
        # engine) so the per-chunk DVE add is a plain 2-byte tensor_tensor
        for h in range(2):
            nc.scalar.activation(
                gsbs[h][:], gss[h][:], mybir.ActivationFunctionType.Copy,
                scale=omd[:],
            )
        pgd = sp.tile([P, 1], F32)  # p_gen = 1 - omd, for the dense scale
        nc.scalar.activation(
            pgd[:], omdp[:], mybir.ActivationFunctionType.Copy,
            bias=1.0, scale=-1.0,
        )

        # ---- the scatter chain (one contiguous GPSIMD block) ----
        for cc in range(NB):
            h, hc = divmod(cc, NBH)
            c, sub = bucket_slot[cc]
            nc.gpsimd.local_scatter(
                out_ap=spts[c][:, sub * BW : (sub + 1) * BW],
                data_ap=gsbs[h][:, hc * S : (hc + 1) * S],
                idxs_ap=meta[:, 2 * TS + cc * S : 2 * TS + (cc + 1) * S],
                channels=P, num_elems=BW, num_idxs=S,
            )

        # ---- per chunk: dense pre-scale -> sparse add -> store ----
        for c, nbk in enumerate(CHUNK_BUCKETS):
            tl, off, cw = tls[c]
            # dense pre-scale by p_gen on the scalar engine (in-place),
            # concurrent with the GPSIMD scatter chain
            nc.scalar.mul(tl[:], tl[:], pgd[:])
            # tl += sparse (already scaled); all-2B tensor_tensor -> 2x DVE
            nc.vector.tensor_tensor(
                tl[:], tl[:], spts[c][:], op=mybir.AluOpType.add
            )
            nc.sync.dma_start(outv[:, off : off + cw], tl[:])


# --------------------------------------------------------------------------
# entry point
# --------------------------------------------------------------------------

last_results = None  # BassKernelResults of the most recent run (for benchmarks)


def build_program(b_const: float, S: int, W: int):
    nc = bacc.Bacc("TRN2", target_bir_lowering=False, debug=False,
                   num_devices=NCORES)
    TS = NB * S
    vd_t = nc.dram_tensor("vd", [BSH * V], BF16, kind="ExternalInput")
    xt_t = nc.dram_tensor("xt", [P, NK * BSH], FP16, kind="ExternalInput")
    wt_t = nc.dram_tensor("wt", [P, NK], FP16, kind="ExternalInput")
    dup_t = nc.dram_tensor("dup", [BSH, P], F32, kind="ExternalInput")
    meta_t = nc.dram_tensor("meta", [P, 3 * TS], I16, kind="ExternalInput")
    out_t = nc.dram_tensor("out", [BSH * V], BF16, kind="ExternalOutput")

    with tile.TileContext(nc) as tc:
        _build_kernel(
            tc,
            out_t.ap(),
            (vd_t.ap(), xt_t.ap(), wt_t.ap(), dup_t.ap(), meta_t.ap()),
            b_const,
            S,
            W,
        )
    nc.compile()
    return nc


def prepare_in_maps(vocab_dist, attn_dist, xcat_full, wall_np, src_ids, vs,
                    S, W):
    dup = np.zeros((BSH, P), np.float32)
    dup[np.arange(P) // 2, np.arange(P)] = 1.0
    wt = np.ascontiguousarray(wall_np.reshape(NK, P).T.astype(np.float16))
    in_maps = []
    for c in range(NCORES):
        sl = slice(c * BSH, (c + 1) * BSH)
        attn_p, ids16, lsidx = _prep_shard(
            attn_dist[sl], src_ids[sl], vs, S, W
        )
        meta = np.ascontiguousarray(
            np.concatenate([ids16, attn_p.view(np.int16), lsidx], axis=1)
        )
        # xT[p, k*BSH + m] = xcat[row m, k*P + p]
        xt = np.ascontiguousarray(
            xcat_full[sl].T.reshape(NK, P, BSH).transpose(1, 0, 2)
            .reshape(P, NK * BSH).astype(np.float16)
        )
        in_maps.append(
            {
                "vd": np.ascontiguousarray(
                    vocab_dist[sl].astype(NP_BF16)
                ).reshape(-1),
                "xt": xt,
                "wt": wt,
                "dup": dup,
                "meta": meta,
            }
        )
    return in_maps


def kernel(vocab_dist, attn_dist, context, state, emb, src_ids, vocab_size,
           w_c, w_s, w_y, b, **kwargs):
    vocab_dist = np.asarray(vocab_dist, dtype=np.float32)
    attn_dist = np.asarray(attn_dist, dtype=np.float32)
    xcat_full = np.ascontiguousarray(
        np.concatenate(
            [np.asarray(context), np.asarray(state), np.asarray(emb)], axis=1
        ).astype(np.float32)
    )
    src_ids = np.asarray(src_ids)
    vs = int(np.asarray(vocab_size))
    wall_np = np.ascontiguousarray(
        np.concatenate(
            [np.asarray(w_c), np.asarray(w_s), np.asarray(w_y)]
        ).astype(np.float32)
    )
    b_const = float(np.asarray(b).reshape(-1)[0])

    assert vocab_dist.shape == (B, V) and attn_dist.shape == (B, T)
    assert xcat_full.shape == (B, D) and src_ids.shape == (B, T)

    # If tracing is requested but this image lacks antenv.axon_hooks,
    # bass_utils would crash on import; provide a no-op hook module.
    try:
        import antenv.axon_hooks  # noqa: F401
    except ImportError:
        import sys
        import types

        try:
            import antenv

            mod = types.ModuleType("antenv.axon_hooks")
            mod.get_axon_ntff_profile_hook = lambda: None
            mod.set_axon_ntff_profile_hook = lambda h: None
            sys.modules["antenv.axon_hooks"] = mod
            antenv.axon_hooks = mod
        except ImportError:
            pass

    S, W = _slot_requirement(src_ids, vs)
    nc = build_program(b_const, S, W)
    in_maps = prepare_in_maps(
        vocab_dist, attn_dist, xcat_full, wall_np, src_ids, vs, S, W
    )

    res = bass_utils.run_bass_kernel_spmd(
        nc, in_maps, core_ids=list(range(NCORES))
    )
    global last_results
    last_results = res

    out = np.empty((B, V), np.float32)
    for c in range(NCORES):
        out[c * BSH : (c + 1) * BSH] = (
            res.results[c]["out"].astype(np.float32).reshape(BSH, V)
        )
    return out
